# revision 27
# baseline (speedup 1.0000x reference)
"""Distributed Trainium2 kernel for the AttentionBlock problem.

Full inputs:
  x     [4, 2048, 512] f32
  w_qkv [512, 1536]    f32   (columns: q | k | v, each 512 wide)
  w_out [512, 512]     f32
  b_out [512]          f32

Sharding over 8 cores: core c handles batch (c // 2) and head-group
(c % 2) -> 4 heads of 64 dims each (feature slice of 256 per section).
Each core computes a partial output projection (its 4 heads' contribution
to out = attn @ w_out); the host sums the two partials per batch and adds
the bias.

Per-core dataflow (bf16 matmuls, all intermediates in SBUF):
  slot (ic, pair, j), i-block of 512 per head, two heads per slot:
    S^T[j 128, i 512x2] = kT.T @ qT    (two matmuls at alternating PE row
                                        positions 0/64 -> LDWEIGHTS of one
                                        overlaps the other's stream)
    P^T = exp(S^T * 0.125)             (one [128,1024] ACT -> SBUF bf16)
    outT[128, 512] += v'_h.T @ P^T_h   (per head; v' is padded to 128
        columns: [v | ones | zeros] so the weight load takes the FWL fast
        path; row 64 of the output accumulates the softmax denominator)
  norm: attnT rows = outT[0:64] * recip(outT[64]) (DVE + gpsimd bcast)
  out_partial = attnT.T @ w_out_shard -> DRAM (f32)

Inputs are host-side rearranged so each SBUF tile loads with ONE dense
DMA (128 descriptors of 1-4KB) spread over the Sync/Scalar/GpSimd queues.
A dummy exp at t0 pulls the ACT table load into the DMA ramp.

Schedule: per slot: exp(i) -> S(i+1) issued immediately -> PV(i-1)
(one slot late so it never waits on the exp that made its P tile) ->
injected projection / norm / output-projection work.  The "qs"-tag PSUM
pool rotates between 2 buffers; injections come in PAIRS per slot so
S(i+1) never lands on the buffer the live exp is reading.  The first
block's phases are interleaved in j-halves (b0a, b1a, b0b, b1b) to
spread the v-projection injections over 32 slots.
"""

import sys

if "/opt/trn_rl_repo" not in sys.path:
    sys.path.insert(0, "/opt/trn_rl_repo")

import numpy as np

DIM = 512
HEADS = 8
DIM_HEAD = 64
INNER = 512
B, N = 4, 2048
N_CORES = 8
HEADS_PER_CORE = 4
FEAT = HEADS_PER_CORE * DIM_HEAD  # 256 features per core per section
SCALE = DIM_HEAD ** -0.5  # 0.125

N_JB = N // 128  # 16 j-blocks
N_WARMUP_MM = 14

_CACHED = {}


def _build():
    import concourse.mybir as mybir
    import concourse.tile as tile
    from concourse import bacc

    f32 = mybir.dt.float32
    bf16 = mybir.dt.bfloat16
    EXP = mybir.ActivationFunctionType.Exp
    MUL = mybir.AluOpType.mult

    nc = bacc.Bacc("TRN2", target_bir_lowering=False, debug=False,
                   num_devices=N_CORES)

    # host-rearranged inputs: one dense DMA per SBUF tile
    xr = nc.declare_dram_parameter("xr", [4, 128, 4, 512], bf16,
                                   isOutput=False)
    wqkr = nc.declare_dram_parameter("wqkr", [4, 128, 4, 128], bf16,
                                     isOutput=False)
    wvr = nc.declare_dram_parameter("wvr", [128, 4, 256], bf16,
                                    isOutput=False)
    wor = nc.declare_dram_parameter("wor", [128, 2, 512], bf16,
                                    isOutput=False)
    # bf16 output: halves the output DMA traffic; the host sums the two
    # partials per batch in f32
    out = nc.declare_dram_parameter("out", [N, DIM], bf16, isOutput=True)

    with tile.TileContext(nc) as tc:
        with (
            tc.tile_pool(name="xb", bufs=1) as xb_pool,
            tc.tile_pool(name="wq", bufs=1) as w_pool,
            tc.tile_pool(name="qkt", bufs=1) as qkt_pool,
            tc.tile_pool(name="vs", bufs=1) as v_pool,
            tc.tile_pool(name="pt", bufs=14) as pt_pool,
            tc.tile_pool(name="attnT", bufs=1) as attnT_pool,
            tc.tile_pool(name="scl", bufs=8) as scl_pool,
            tc.tile_pool(name="dout", bufs=4) as dout_pool,
            tc.tile_pool(name="warm", bufs=1) as warm_pool,
            tc.tile_pool(name="mm", bufs=2, space="PSUM") as mm_psum,
            tc.tile_pool(name="pv", bufs=4, space="PSUM") as pv_psum,
        ):
            # ---- tiny dummy activation first: pulls the exp table load
            # (~1.3us) into the DMA ramp ----
            wsrc = warm_pool.tile([128, 512], bf16, tag="wsrc", name="wsrc")
            nc.vector.memset(wsrc[:], 0.0)
            dumm = warm_pool.tile([1, 1], f32, tag="dumm", name="dumm")
            nc.scalar.activation(dumm[:], wsrc[0:1, 0:1], EXP)

            # ---- PE warmup: keep the HAM clock gate busy during the DMA
            # ramp so real matmuls start at full rate ----
            wps = mm_psum.tile([128, 512], f32, tag="qs", name="warmps")
            with nc.named_scope("warmup"):
                for _ in range(N_WARMUP_MM):
                    nc.tensor.matmul(wps[:], wsrc[:, 0:128], wsrc[:],
                                     start=True, stop=True)

            # ---- DMAs spread across the Sync / Scalar / GpSimd queues
            # (the only DMA-capable ones).  A queue sustains ~38 GB/s, so
            # big tiles are split into 128-256KB pieces and balanced across
            # the three queues in need order: x0 + the first weight slices
            # gate the first exp (~12us); x1/x2/x3 land just before the
            # k-projections that need them. ----
            qS, qA, qG = nc.sync, nc.scalar, nc.gpsimd

            xb = [None] * 4  # [n] -> [128, 4, 512] bf16 (k in dim 1)

            def xtile(n):
                t = xb_pool.tile([128, 4, 512], bf16, tag=f"xb{n}",
                                 name=f"xb{n}")
                xb[n] = t
                return t

            def dma_x(n, klo, khi, q):
                q.dma_start(out=xb[n][:, klo:khi, :], in_=xr[n][:, klo:khi])

            # wqk m-slices: m 0/1 = q heads 01 / 23; m 2/3 = k heads 01 / 23
            wqk_t = w_pool.tile([128, 4, 4, 128], bf16, tag="wqk",
                                name="wqkb")

            def dma_wqk(m, q):
                q.dma_start(out=wqk_t[:, m], in_=wqkr[m])

            wv_t = w_pool.tile([128, 4, 256], bf16, tag="wv", name="wvb")
            wo = w_pool.tile([128, 2, 512], bf16, tag="wo", name="wob")
            for n in range(4):
                xtile(n)

            # need-order, ~38GB/s per queue:
            #  S: x0k0(7) x0k1(10) x1k01(17) wv23(20) x2k01(27)
            #  A: m0(7) m2(10) m1(14) m3(17) x2k23(24) x3k23(31)
            #  G: x0k2(7) x0k3(10) x1k23(17) wv01(24) x3k01(30) wo(37)
            dma_x(0, 0, 1, qS)
            dma_wqk(0, qA)         # q heads 01 (first proj)
            dma_x(0, 2, 3, qG)
            dma_x(0, 1, 2, qS)
            dma_wqk(2, qA)         # k heads 01 (first S)
            dma_x(0, 3, 4, qG)
            dma_wqk(1, qA)         # q heads 23 (P(1,0) at slot 2)
            dma_x(1, 0, 2, qS)
            dma_x(1, 2, 4, qG)
            dma_wqk(3, qA)         # k heads 23 (P(3,0) at slot 2)
            qS.dma_start(out=wv_t[:, 2:4, :], in_=wvr[:, 2:4])
            qG.dma_start(out=wv_t[:, 0:2, :], in_=wvr[:, 0:2])
            dma_x(2, 2, 4, qA)
            dma_x(2, 0, 2, qS)
            dma_x(3, 0, 2, qG)
            dma_x(3, 2, 4, qA)
            qG.dma_start(out=wo[:], in_=wor[:])

            # ---- persistent SBUF tiles ----
            qkt = [qkt_pool.tile([128, N], bf16, tag=f"qkt{m}", name=f"qkt{m}")
                   for m in range(4)]
            # v tiles padded to 128 cols per head: [v (64) | ones | zeros]
            vt = [v_pool.tile([128, 4, 128], bf16, tag=f"v{j}", name=f"v{j}")
                  for j in range(N_JB)]
            attnT = [attnT_pool.tile([128, N], bf16, tag=f"aT{t}",
                                     name=f"aT{t}")
                     for t in range(2)]

            def proj_qk(m, n, c0=0, cw=512):
                """qkt[m][:, n*512+c0 : +cw] from x block n."""
                ncol = slice(n * 512 + c0, n * 512 + c0 + cw)
                with nc.named_scope("proj"):
                    ps = mm_psum.tile([128, cw], f32, tag="qs", name="psb")
                    for k in range(4):
                        nc.tensor.matmul(
                            ps[:],
                            wqk_t[:, m, k, :],
                            xb[n][:, k, c0:c0 + cw],
                            start=(k == 0), stop=(k == 3),
                        )
                    nc.vector.tensor_copy(qkt[m][:, ncol], ps[:])

            def proj_v(j):
                n, jj = j // 4, j % 4
                with nc.named_scope("proj"):
                    ps = mm_psum.tile([128, 256], f32, tag="qs", name="psv")
                    for k in range(4):
                        nc.tensor.matmul(
                            ps[:],
                            xb[n][:, k, jj * 128:(jj + 1) * 128],
                            wv_t[:, k, :],
                            start=(k == 0), stop=(k == 3),
                        )
                    nc.vector.tensor_copy(
                        vt[j][:, :, 0:64], ps.rearrange("p (h f) -> p h f", h=4)
                    )
                    nc.vector.memset(vt[j][:, :, 64:65], 1.0)
                    nc.vector.memset(vt[j][:, :, 65:128], 0.0)

            def normalize(ic, pair, hh):
                """attnT rows for (pair, hh), cols ic*512 : +512."""
                pv_ps = outps[(ic, pair)][hh]
                i0 = ic * 512
                with nc.named_scope("norm"):
                    # copy the denominator row to SBUF first: a PSUM-sourced
                    # reciprocal_approx_fast reads the wrong partition
                    lrow = scl_pool.tile([1, 512], f32, tag="lrow", name="lrow")
                    nc.vector.tensor_copy(lrow[:], pv_ps[64:65, :])
                    rl = scl_pool.tile([1, 512], f32, tag="rl", name="rl")
                    nc.vector.reciprocal_approx_fast(rl[:], lrow[:])
                    rlb = scl_pool.tile([64, 512], f32, tag="rlb", name="rlb")
                    nc.gpsimd.partition_broadcast(rlb[:], rl[:])
                    nc.vector.tensor_tensor(
                        attnT[pair][hh * 64:(hh + 1) * 64, i0:i0 + 512],
                        pv_ps[0:64, :], rlb[:], MUL,
                    )

            def outproj_chunk(chunk, q=None):
                with nc.named_scope("outproj"):
                    ps = mm_psum.tile([128, 512], f32, tag="qs", name="psd")
                    for t in range(2):
                        nc.tensor.matmul(
                            ps[:],
                            attnT[t][:, chunk * 128:(chunk + 1) * 128],
                            wo[:, t, :],
                            start=(t == 0), stop=(t == 1),
                        )
                    ot = dout_pool.tile([128, 512], bf16, tag="ot", name="ot")
                    nc.vector.tensor_copy(ot[:], ps[:])
                    # spread output DMAs over the sync/gpsimd queues (the
                    # scalar queue is kept clean while exps are running)
                    if q is None:
                        q = nc.sync if chunk % 2 == 0 else nc.gpsimd
                    q.dma_start(out=out[chunk * 128:(chunk + 1) * 128, :],
                                in_=ot[:])

            # ---- the attention slot stream ----
            slots = []

            def phase(ic, pair, jlo, jhi):
                for j in range(jlo, jhi):
                    slots.append(dict(ic=ic, pair=pair, j=j))

            phase(0, 0, 0, 4)     # slots 0-3
            phase(0, 1, 0, 4)     # slots 4-7
            phase(0, 0, 4, 8)     # slots 8-11
            phase(0, 1, 4, 8)     # slots 12-15
            phase(0, 0, 8, 16)    # slots 16-23
            phase(0, 1, 8, 16)    # slots 24-31
            for ic in range(1, 4):
                phase(ic, 0, 0, 16)
                phase(ic, 1, 0, 16)

            # PV flush plan: PVs accumulate in a FIFO; nothing is flushed
            # for the first 11 slots (the v weights arrive ~24us), then
            # 3/slot until the lag settles at 1.  A late PV never waits on
            # the exp that made its P tile.  pt bufs (14) exceeds the max
            # lag (11) so exp never clobbers an unread P tile.
            pend = []
            n_flush = [0] * 11 + [3] * 6 + [1] * 200

            def flush_pv(k):
                for _ in range(min(k, len(pend))):
                    pend.pop(0)()

            outps = {}

            def issue_S(s):
                ic, pair, j = s["ic"], s["pair"], s["j"]
                qt, kt = qkt[pair], qkt[2 + pair]
                i0 = ic * 512
                with nc.named_scope("attnS"):
                    qs = mm_psum.tile([128, 1024], f32, tag="qs", name="qs")
                    for hh in range(2):
                        nc.tensor.matmul(
                            qs[:, hh * 512:(hh + 1) * 512],
                            kt[hh * 64:(hh + 1) * 64, j * 128:(j + 1) * 128],
                            qt[hh * 64:(hh + 1) * 64, i0:i0 + 512],
                            start=True, stop=True,
                        )
                s["qs"] = qs

            def run_slots(inject):
                issue_S(slots[0])
                for i, s in enumerate(slots):
                    with nc.named_scope("exp"):
                        p = pt_pool.tile([128, 1024], bf16, tag="pt",
                                         name="ptile")
                        nc.scalar.activation(p[:], s["qs"][:], EXP,
                                             scale=SCALE)
                    if i + 1 < len(slots):
                        issue_S(slots[i + 1])
                    flush_pv(n_flush[i])
                    ic, pair, j = s["ic"], s["pair"], s["j"]
                    key = (ic, pair)
                    if key not in outps:
                        outps[key] = [
                            pv_psum.tile([128, 512], f32, tag="pv",
                                         name=f"o{ic}{pair}{hh}")
                            for hh in range(2)
                        ]
                    outp = outps[key]

                    def pv(p=p, pair=pair, j=j, outp=outp):
                        with nc.named_scope("pv"):
                            for hh in range(2):
                                h = 2 * pair + hh
                                nc.tensor.matmul(
                                    outp[hh][:],
                                    vt[j][:, h, :],
                                    p[:, hh * 512:(hh + 1) * 512],
                                    start=(j == 0),
                                    stop=(j == N_JB - 1),
                                )
                    pend.append(pv)
                    for t in inject.get(i, ()):
                        t()

            P = proj_qk
            V = proj_v

            def nt(ic, pair, hh):
                return lambda: normalize(ic, pair, hh)

            def oc(c):
                return lambda: outproj_chunk(c)

            # ---- injection schedule ----
            # qs-tag psum allocs per slot must be EVEN so S(i+1) never
            # rotates onto the buffer the live exp(i) is reading.  Units:
            # P/V/outproj alloc 1 each (paired); norms alloc none.
            # k-proj cols for j must be injected (program order) before the
            # S that reads them is issued; V(j) before PV(j)'s flush; and
            # DMA-gated work sits at slots the PE reaches only after the
            # data lands (else the in-order PE FIFO stalls behind it).
            inj = {
                0: [lambda: P(2, 0, 256, 128), lambda: P(2, 0, 384, 128)],
                2: [lambda: P(1, 0), lambda: P(3, 0)],
                6: [lambda: P(2, 1, 0, 128), lambda: P(2, 1, 128, 128)],
                7: [lambda: P(2, 1, 256, 128), lambda: P(2, 1, 384, 128)],
                8: [lambda: P(3, 1), lambda: P(0, 1)],
                9: [lambda: V(0), lambda: V(1)],
                10: [lambda: V(2), lambda: V(3)],
                11: [lambda: V(4), lambda: V(5)],
                12: [lambda: V(6), lambda: V(7)],
                13: [lambda: P(2, 2, 0, 256), lambda: P(2, 2, 256, 256)],
                15: [lambda: V(8), lambda: V(9)],
                16: [lambda: V(10), lambda: V(11)],
                17: [lambda: P(2, 3, 0, 256), lambda: P(2, 3, 256, 256)],
                19: [lambda: P(3, 2), lambda: P(1, 1)],
                20: [lambda: V(12), lambda: V(13)],
                21: [lambda: V(14), lambda: V(15)],
                24: [lambda: P(3, 3), lambda: P(0, 2)],
                25: [nt(0, 0, 0), nt(0, 0, 1)],
                33: [nt(0, 1, 0), nt(0, 1, 1)],
                36: [oc(0), oc(1)],
                40: [oc(2), oc(3)],
                44: [lambda: P(0, 3), lambda: P(1, 2)],
                48: [lambda: P(1, 3)],  # odd singleton: quiet region
                49: [nt(1, 0, 0), nt(1, 0, 1)],
                65: [nt(1, 1, 0), nt(1, 1, 1)],
                68: [oc(4), oc(5)],
                72: [oc(6), oc(7)],
                81: [nt(2, 0, 0), nt(2, 0, 1)],
                97: [nt(2, 1, 0), nt(2, 1, 1)],
                100: [oc(8), oc(9)],
                104: [oc(10), oc(11)],
                113: [nt(3, 0, 0), nt(3, 0, 1)],
            }

            # ---- prologue: minimum work before the first exp ----
            P(0, 0)            # q heads 01, i 0:512
            P(2, 0, 0, 256)    # k heads 01, j 0:256 (slots 0..1)

            run_slots(inj)

            # ---- tail: flush the last PVs, final norms, last 4 chunks
            # (output DMAs fan out across all three queues; the scalar
            # queue is free once the exps are done) ----
            flush_pv(10)
            normalize(3, 1, 0)
            normalize(3, 1, 1)
            for c, q in zip(range(12, 16),
                            [nc.sync, nc.gpsimd, nc.scalar, nc.sync]):
                outproj_chunk(c, q)

    nc.compile()
    return nc


def _get_nc():
    if "nc" not in _CACHED:
        _CACHED["nc"] = _build()
    return _CACHED["nc"]


def kernel(x, w_qkv, w_out, b_out):
    import concourse.mybir as mybir
    from concourse.bass_utils import run_bass_kernel_spmd

    bf16 = mybir.dt.np(mybir.dt.bfloat16)

    x = np.asarray(x, dtype=np.float32)
    w_qkv = np.asarray(w_qkv, dtype=np.float32)
    w_out = np.asarray(w_out, dtype=np.float32)
    b_out = np.asarray(b_out, dtype=np.float32)

    in_maps = []
    for c in range(N_CORES):
        bi, hg = c // 2, c % 2
        f0 = hg * FEAT
        wq = w_qkv[:, f0:f0 + FEAT]
        wk = w_qkv[:, INNER + f0:INNER + f0 + FEAT]
        wvs = w_qkv[:, 2 * INNER + f0:2 * INNER + f0 + FEAT]

        # x: [n, d] -> xr[nb, p, k, c] = x[nb*512+c, k*128+p]
        xT = np.ascontiguousarray(x[bi].T)  # [d, n]
        xr = xT.reshape(4, 128, 4, 512).transpose(2, 1, 0, 3)

        # wqk: [d, f] (q|k) -> wqkr[m, p, k, c] = wqk[k*128+p, m*128+c]
        wqk = np.concatenate([wq, wk], axis=1)  # [512, 512]
        wqkr = wqk.reshape(4, 128, 4, 128).transpose(2, 1, 0, 3)

        # wv: [d, f] -> wvr[p, k, c] = wv[k*128+p, c]
        wvr = wvs.reshape(4, 128, 256).transpose(1, 0, 2)

        # w_out shard: [f, dim] -> wor[p, t, c] = w_out[t*128+p, c]
        wos = w_out[f0:f0 + FEAT, :]
        wor = wos.reshape(2, 128, 512).transpose(1, 0, 2)

        in_maps.append({
            "xr": np.ascontiguousarray(xr).astype(bf16),
            "wqkr": np.ascontiguousarray(wqkr).astype(bf16),
            "wvr": np.ascontiguousarray(wvr).astype(bf16),
            "wor": np.ascontiguousarray(wor).astype(bf16),
        })

    nc = _get_nc()
    res = run_bass_kernel_spmd(nc, in_maps, list(range(N_CORES)))

    outa = np.empty((B, N, DIM), dtype=np.float32)
    for bi in range(B):
        outa[bi] = (res.results[2 * bi]["out"].astype(np.float32)
                    + res.results[2 * bi + 1]["out"].astype(np.float32)
                    + b_out)
    return outa


# revision 37
# speedup vs baseline: 1.0037x; 1.0037x over previous
"""Distributed Trainium2 kernel for the AttentionBlock problem.

Full inputs:
  x     [4, 2048, 512] f32
  w_qkv [512, 1536]    f32   (columns: q | k | v, each 512 wide)
  w_out [512, 512]     f32
  b_out [512]          f32

Sharding over 8 cores: core c handles batch (c // 2) and head-group
(c % 2) -> 4 heads of 64 dims each (feature slice of 256 per section).
Each core computes a partial output projection (its 4 heads' contribution
to out = attn @ w_out); the host sums the two partials per batch and adds
the bias.

Per-core dataflow (bf16 matmuls, all intermediates in SBUF):
  slot (ic, pair, j), i-block of 512 per head, two heads per slot:
    S^T[j 128, i 512x2] = kT.T @ qT    (two matmuls at alternating PE row
                                        positions 0/64 -> LDWEIGHTS of one
                                        overlaps the other's stream)
    P^T = exp(S^T * 0.125)             (one [128,1024] ACT -> SBUF bf16)
    outT[128, 512] += v'_h.T @ P^T_h   (per head; v' is padded to 128
        columns: [v | ones | zeros] so the weight load takes the FWL fast
        path; row 64 of the output accumulates the softmax denominator)
  norm: attnT rows = outT[0:64] * recip(outT[64]) (DVE + gpsimd bcast)
  out_partial = attnT.T @ w_out_shard -> DRAM (f32)

Inputs are host-side rearranged so each SBUF tile loads with ONE dense
DMA (128 descriptors of 1-4KB) spread over the Sync/Scalar/GpSimd queues.
A dummy exp at t0 pulls the ACT table load into the DMA ramp.

Schedule: per slot: exp(i) -> S(i+1) issued immediately -> PV(i-1)
(one slot late so it never waits on the exp that made its P tile) ->
injected projection / norm / output-projection work.  The "qs"-tag PSUM
pool rotates between 2 buffers; injections come in PAIRS per slot so
S(i+1) never lands on the buffer the live exp is reading.  The first
block's phases are interleaved in j-halves (b0a, b1a, b0b, b1b) to
spread the v-projection injections over 32 slots.
"""

import sys

if "/opt/trn_rl_repo" not in sys.path:
    sys.path.insert(0, "/opt/trn_rl_repo")

import numpy as np

DIM = 512
HEADS = 8
DIM_HEAD = 64
INNER = 512
B, N = 4, 2048
N_CORES = 8
HEADS_PER_CORE = 4
FEAT = HEADS_PER_CORE * DIM_HEAD  # 256 features per core per section
SCALE = DIM_HEAD ** -0.5  # 0.125

N_JB = N // 128  # 16 j-blocks
N_WARMUP_MM = 14

_CACHED = {}


def _build():
    import concourse.mybir as mybir
    import concourse.tile as tile
    from concourse import bacc

    f32 = mybir.dt.float32
    bf16 = mybir.dt.bfloat16
    EXP = mybir.ActivationFunctionType.Exp
    MUL = mybir.AluOpType.mult

    nc = bacc.Bacc("TRN2", target_bir_lowering=False, debug=False,
                   num_devices=N_CORES)

    # host-rearranged inputs: one dense DMA per SBUF tile
    xr = nc.declare_dram_parameter("xr", [4, 128, 4, 512], bf16,
                                   isOutput=False)
    wqkr = nc.declare_dram_parameter("wqkr", [4, 128, 4, 128], bf16,
                                     isOutput=False)
    wvr = nc.declare_dram_parameter("wvr", [128, 4, 256], bf16,
                                    isOutput=False)
    wor = nc.declare_dram_parameter("wor", [128, 2, 512], bf16,
                                    isOutput=False)
    # bf16 output: halves the output DMA traffic; the host sums the two
    # partials per batch in f32
    out = nc.declare_dram_parameter("out", [N, DIM], bf16, isOutput=True)

    with tile.TileContext(nc) as tc:
        with (
            tc.tile_pool(name="xb", bufs=1) as xb_pool,
            tc.tile_pool(name="wq", bufs=1) as w_pool,
            tc.tile_pool(name="qkt", bufs=1) as qkt_pool,
            tc.tile_pool(name="vs", bufs=1) as v_pool,
            tc.tile_pool(name="pt", bufs=14) as pt_pool,
            tc.tile_pool(name="attnT", bufs=1) as attnT_pool,
            tc.tile_pool(name="scl", bufs=8) as scl_pool,
            tc.tile_pool(name="dout", bufs=4) as dout_pool,
            tc.tile_pool(name="warm", bufs=1) as warm_pool,
            tc.tile_pool(name="mm", bufs=2, space="PSUM") as mm_psum,
            tc.tile_pool(name="pv", bufs=4, space="PSUM") as pv_psum,
        ):
            # ---- tiny dummy activation first: pulls the exp table load
            # (~1.3us) into the DMA ramp ----
            wsrc = warm_pool.tile([128, 512], bf16, tag="wsrc", name="wsrc")
            nc.vector.memset(wsrc[:], 0.0)
            dumm = warm_pool.tile([1, 1], f32, tag="dumm", name="dumm")
            nc.scalar.activation(dumm[:], wsrc[0:1, 0:1], EXP)

            # ---- PE warmup: keep the HAM clock gate busy during the DMA
            # ramp so real matmuls start at full rate ----
            wps = mm_psum.tile([128, 512], f32, tag="qs", name="warmps")
            with nc.named_scope("warmup"):
                for _ in range(N_WARMUP_MM):
                    nc.tensor.matmul(wps[:], wsrc[:, 0:128], wsrc[:],
                                     start=True, stop=True)

            # ---- DMAs spread across the Sync / Scalar / GpSimd queues
            # (the only DMA-capable ones).  A queue sustains ~38 GB/s, so
            # big tiles are split into 128-256KB pieces and balanced across
            # the three queues in need order: x0 + the first weight slices
            # gate the first exp (~12us); x1/x2/x3 land just before the
            # k-projections that need them. ----
            qS, qA, qG = nc.sync, nc.scalar, nc.gpsimd

            xb = [None] * 4  # [n] -> [128, 4, 512] bf16 (k in dim 1)

            def xtile(n):
                t = xb_pool.tile([128, 4, 512], bf16, tag=f"xb{n}",
                                 name=f"xb{n}")
                xb[n] = t
                return t

            def dma_x(n, klo, khi, q):
                q.dma_start(out=xb[n][:, klo:khi, :], in_=xr[n][:, klo:khi])

            # wqk m-slices: m 0/1 = q heads 01 / 23; m 2/3 = k heads 01 / 23
            wqk_t = w_pool.tile([128, 4, 4, 128], bf16, tag="wqk",
                                name="wqkb")

            def dma_wqk(m, q):
                q.dma_start(out=wqk_t[:, m], in_=wqkr[m])

            wv_t = w_pool.tile([128, 4, 256], bf16, tag="wv", name="wvb")
            wo = w_pool.tile([128, 2, 512], bf16, tag="wo", name="wob")
            for n in range(4):
                xtile(n)

            # need-order, ~38GB/s per queue:
            #  S: x0k0(7) x0k1(10) m3(14) x1k01(20) x2k01(27)
            #  A: m0(7) m2(10) m1(14) wv23(20) x2k23(27) x3k23(34)
            #  G: x0k2(7) x0k3(10) x1k23(17) wv01(24) x3k01(30) wo(37)
            dma_x(0, 0, 1, qS)
            dma_wqk(0, qA)         # q heads 01 (first proj)
            dma_x(0, 2, 3, qG)
            dma_x(0, 1, 2, qS)
            dma_wqk(2, qA)         # k heads 01 (first S)
            dma_x(0, 3, 4, qG)
            dma_wqk(3, qS)         # k heads 23 (P(3,0) at slot 2)
            dma_wqk(1, qA)         # q heads 23 (P(1,0) at slot 2)
            dma_x(1, 2, 4, qG)
            dma_x(1, 0, 2, qS)
            qA.dma_start(out=wv_t[:, 2:4, :], in_=wvr[:, 2:4])
            qG.dma_start(out=wv_t[:, 0:2, :], in_=wvr[:, 0:2])
            dma_x(2, 0, 2, qS)
            dma_x(2, 2, 4, qA)
            dma_x(3, 0, 2, qG)
            dma_x(3, 2, 4, qA)
            qG.dma_start(out=wo[:], in_=wor[:])

            # ---- persistent SBUF tiles ----
            qkt = [qkt_pool.tile([128, N], bf16, tag=f"qkt{m}", name=f"qkt{m}")
                   for m in range(4)]
            # v tiles padded to 128 cols per head: [v (64) | ones | zeros]
            vt = [v_pool.tile([128, 4, 128], bf16, tag=f"v{j}", name=f"v{j}")
                  for j in range(N_JB)]
            attnT = [attnT_pool.tile([128, N], bf16, tag=f"aT{t}",
                                     name=f"aT{t}")
                     for t in range(2)]

            def proj_qk(m, n, c0=0, cw=512):
                """qkt[m][:, n*512+c0 : +cw] from x block n."""
                ncol = slice(n * 512 + c0, n * 512 + c0 + cw)
                with nc.named_scope("proj"):
                    ps = mm_psum.tile([128, cw], f32, tag="qs", name="psb")
                    for k in range(4):
                        nc.tensor.matmul(
                            ps[:],
                            wqk_t[:, m, k, :],
                            xb[n][:, k, c0:c0 + cw],
                            start=(k == 0), stop=(k == 3),
                        )
                    nc.vector.tensor_copy(qkt[m][:, ncol], ps[:])

            def proj_v(j):
                n, jj = j // 4, j % 4
                with nc.named_scope("proj"):
                    ps = mm_psum.tile([128, 256], f32, tag="qs", name="psv")
                    for k in range(4):
                        nc.tensor.matmul(
                            ps[:],
                            xb[n][:, k, jj * 128:(jj + 1) * 128],
                            wv_t[:, k, :],
                            start=(k == 0), stop=(k == 3),
                        )
                    nc.vector.tensor_copy(
                        vt[j][:, :, 0:64], ps.rearrange("p (h f) -> p h f", h=4)
                    )
                    nc.vector.memset(vt[j][:, :, 64:65], 1.0)
                    nc.vector.memset(vt[j][:, :, 65:128], 0.0)

            def normalize(ic, pair, hh):
                """attnT rows for (pair, hh), cols ic*512 : +512."""
                pv_ps = outps[(ic, pair)][hh]
                i0 = ic * 512
                with nc.named_scope("norm"):
                    # copy the denominator row to SBUF first: a PSUM-sourced
                    # reciprocal_approx_fast reads the wrong partition
                    lrow = scl_pool.tile([1, 512], f32, tag="lrow", name="lrow")
                    nc.vector.tensor_copy(lrow[:], pv_ps[64:65, :])
                    rl = scl_pool.tile([1, 512], f32, tag="rl", name="rl")
                    nc.vector.reciprocal_approx_fast(rl[:], lrow[:])
                    rlb = scl_pool.tile([64, 512], f32, tag="rlb", name="rlb")
                    nc.gpsimd.partition_broadcast(rlb[:], rl[:])
                    nc.vector.tensor_tensor(
                        attnT[pair][hh * 64:(hh + 1) * 64, i0:i0 + 512],
                        pv_ps[0:64, :], rlb[:], MUL,
                    )

            def outproj_chunk(chunk, q=None):
                with nc.named_scope("outproj"):
                    ps = mm_psum.tile([128, 512], f32, tag="qs", name="psd")
                    for t in range(2):
                        nc.tensor.matmul(
                            ps[:],
                            attnT[t][:, chunk * 128:(chunk + 1) * 128],
                            wo[:, t, :],
                            start=(t == 0), stop=(t == 1),
                        )
                    ot = dout_pool.tile([128, 512], bf16, tag="ot", name="ot")
                    nc.vector.tensor_copy(ot[:], ps[:])
                    # spread output DMAs over the sync/gpsimd queues (the
                    # scalar queue is kept clean while exps are running)
                    if q is None:
                        q = nc.sync if chunk % 2 == 0 else nc.gpsimd
                    q.dma_start(out=out[chunk * 128:(chunk + 1) * 128, :],
                                in_=ot[:])

            # ---- the attention slot stream ----
            slots = []

            def phase(ic, pair, jlo, jhi):
                for j in range(jlo, jhi):
                    slots.append(dict(ic=ic, pair=pair, j=j))

            phase(0, 0, 0, 4)     # slots 0-3
            phase(0, 1, 0, 4)     # slots 4-7
            phase(0, 0, 4, 8)     # slots 8-11
            phase(0, 1, 4, 8)     # slots 12-15
            phase(0, 0, 8, 16)    # slots 16-23
            phase(0, 1, 8, 16)    # slots 24-31
            for ic in range(1, 4):
                phase(ic, 0, 0, 16)
                phase(ic, 1, 0, 16)

            # PV flush plan: PVs accumulate in a FIFO; nothing is flushed
            # for the first 11 slots (the v weights arrive ~24us), then
            # 3/slot until the lag settles at 1.  A late PV never waits on
            # the exp that made its P tile.  pt bufs (14) exceeds the max
            # lag (11) so exp never clobbers an unread P tile.
            pend = []
            n_flush = [0] * 11 + [3] * 6 + [1] * 200

            def flush_pv(k):
                for _ in range(min(k, len(pend))):
                    pend.pop(0)()

            outps = {}

            def issue_S(s):
                ic, pair, j = s["ic"], s["pair"], s["j"]
                qt, kt = qkt[pair], qkt[2 + pair]
                i0 = ic * 512
                with nc.named_scope("attnS"):
                    qs = mm_psum.tile([128, 1024], f32, tag="qs", name="qs")
                    for hh in range(2):
                        nc.tensor.matmul(
                            qs[:, hh * 512:(hh + 1) * 512],
                            kt[hh * 64:(hh + 1) * 64, j * 128:(j + 1) * 128],
                            qt[hh * 64:(hh + 1) * 64, i0:i0 + 512],
                            start=True, stop=True,
                        )
                s["qs"] = qs

            def run_slots(inject):
                issue_S(slots[0])
                for i, s in enumerate(slots):
                    with nc.named_scope("exp"):
                        p = pt_pool.tile([128, 1024], bf16, tag="pt",
                                         name="ptile")
                        nc.scalar.activation(p[:], s["qs"][:], EXP,
                                             scale=SCALE)
                    if i + 1 < len(slots):
                        issue_S(slots[i + 1])
                    flush_pv(n_flush[i])
                    ic, pair, j = s["ic"], s["pair"], s["j"]
                    key = (ic, pair)
                    if key not in outps:
                        outps[key] = [
                            pv_psum.tile([128, 512], f32, tag="pv",
                                         name=f"o{ic}{pair}{hh}")
                            for hh in range(2)
                        ]
                    outp = outps[key]

                    def pv(p=p, pair=pair, j=j, outp=outp):
                        with nc.named_scope("pv"):
                            for hh in range(2):
                                h = 2 * pair + hh
                                nc.tensor.matmul(
                                    outp[hh][:],
                                    vt[j][:, h, :],
                                    p[:, hh * 512:(hh + 1) * 512],
                                    start=(j == 0),
                                    stop=(j == N_JB - 1),
                                )
                    pend.append(pv)
                    for t in inject.get(i, ()):
                        t()

            P = proj_qk
            V = proj_v

            def nt(ic, pair, hh):
                return lambda: normalize(ic, pair, hh)

            def oc(c):
                return lambda: outproj_chunk(c)

            # ---- injection schedule ----
            # qs-tag psum allocs per slot must be EVEN so S(i+1) never
            # rotates onto the buffer the live exp(i) is reading.  Units:
            # P/V/outproj alloc 1 each (paired); norms alloc none.
            # k-proj cols for j must be injected (program order) before the
            # S that reads them is issued; V(j) before PV(j)'s flush; and
            # DMA-gated work sits at slots the PE reaches only after the
            # data lands (else the in-order PE FIFO stalls behind it).
            inj = {
                0: [lambda: P(2, 0, 256, 128), lambda: P(2, 0, 384, 128)],
                2: [lambda: P(1, 0), lambda: P(3, 0)],
                6: [lambda: P(2, 1, 0, 256), lambda: P(2, 1, 256, 256)],
                8: [lambda: P(3, 1), lambda: P(0, 1)],
                9: [lambda: V(0), lambda: V(1)],
                10: [lambda: V(2), lambda: V(3)],
                11: [lambda: V(4), lambda: V(5)],
                12: [lambda: V(6), lambda: V(7)],
                13: [lambda: P(2, 2, 0, 256), lambda: P(2, 2, 256, 256)],
                15: [lambda: V(8), lambda: V(9)],
                16: [lambda: V(10), lambda: V(11)],
                17: [lambda: P(2, 3, 0, 256), lambda: P(2, 3, 256, 256)],
                19: [lambda: P(3, 2), lambda: P(1, 1)],
                20: [lambda: V(12), lambda: V(13)],
                21: [lambda: V(14), lambda: V(15)],
                24: [lambda: P(3, 3), lambda: P(0, 2)],
                25: [nt(0, 0, 0), nt(0, 0, 1)],
                33: [nt(0, 1, 0), nt(0, 1, 1)],
                36: [oc(0), oc(1)],
                40: [oc(2), oc(3)],
                44: [lambda: P(0, 3), lambda: P(1, 2)],
                48: [lambda: P(1, 3)],  # odd singleton: quiet region
                49: [nt(1, 0, 0), nt(1, 0, 1)],
                65: [nt(1, 1, 0), nt(1, 1, 1)],
                68: [oc(4), oc(5)],
                72: [oc(6), oc(7)],
                81: [nt(2, 0, 0), nt(2, 0, 1)],
                97: [nt(2, 1, 0), nt(2, 1, 1)],
                100: [oc(8), oc(9)],
                104: [oc(10), oc(11)],
                113: [nt(3, 0, 0), nt(3, 0, 1)],
            }

            # ---- prologue: minimum work before the first exp ----
            P(0, 0)            # q heads 01, i 0:512
            P(2, 0, 0, 256)    # k heads 01, j 0:256 (slots 0..1)

            run_slots(inj)

            # ---- tail: flush the last PVs, final norms, last 4 chunks
            # (output DMAs fan out across all three queues; the scalar
            # queue is free once the exps are done) ----
            flush_pv(10)
            normalize(3, 1, 0)
            normalize(3, 1, 1)
            for c, q in zip(range(12, 16),
                            [nc.sync, nc.gpsimd, nc.scalar, nc.sync]):
                outproj_chunk(c, q)

    nc.compile()
    return nc


def _get_nc():
    if "nc" not in _CACHED:
        _CACHED["nc"] = _build()
    return _CACHED["nc"]


def kernel(x, w_qkv, w_out, b_out):
    import concourse.mybir as mybir
    from concourse.bass_utils import run_bass_kernel_spmd

    bf16 = mybir.dt.np(mybir.dt.bfloat16)

    x = np.asarray(x, dtype=np.float32)
    w_qkv = np.asarray(w_qkv, dtype=np.float32)
    w_out = np.asarray(w_out, dtype=np.float32)
    b_out = np.asarray(b_out, dtype=np.float32)

    in_maps = []
    for c in range(N_CORES):
        bi, hg = c // 2, c % 2
        f0 = hg * FEAT
        wq = w_qkv[:, f0:f0 + FEAT]
        wk = w_qkv[:, INNER + f0:INNER + f0 + FEAT]
        wvs = w_qkv[:, 2 * INNER + f0:2 * INNER + f0 + FEAT]

        # x: [n, d] -> xr[nb, p, k, c] = x[nb*512+c, k*128+p]
        xT = np.ascontiguousarray(x[bi].T)  # [d, n]
        xr = xT.reshape(4, 128, 4, 512).transpose(2, 1, 0, 3)

        # wqk: [d, f] (q|k) -> wqkr[m, p, k, c] = wqk[k*128+p, m*128+c]
        wqk = np.concatenate([wq, wk], axis=1)  # [512, 512]
        wqkr = wqk.reshape(4, 128, 4, 128).transpose(2, 1, 0, 3)

        # wv: [d, f] -> wvr[p, k, c] = wv[k*128+p, c]
        wvr = wvs.reshape(4, 128, 256).transpose(1, 0, 2)

        # w_out shard: [f, dim] -> wor[p, t, c] = w_out[t*128+p, c]
        wos = w_out[f0:f0 + FEAT, :]
        wor = wos.reshape(2, 128, 512).transpose(1, 0, 2)

        in_maps.append({
            "xr": np.ascontiguousarray(xr).astype(bf16),
            "wqkr": np.ascontiguousarray(wqkr).astype(bf16),
            "wvr": np.ascontiguousarray(wvr).astype(bf16),
            "wor": np.ascontiguousarray(wor).astype(bf16),
        })

    nc = _get_nc()
    res = run_bass_kernel_spmd(nc, in_maps, list(range(N_CORES)))

    outa = np.empty((B, N, DIM), dtype=np.float32)
    for bi in range(B):
        outa[bi] = (res.results[2 * bi]["out"].astype(np.float32)
                    + res.results[2 * bi + 1]["out"].astype(np.float32)
                    + b_out)
    return outa
